# revision 2
# baseline (speedup 1.0000x reference)
"""KANLinear forward on 8 Trainium2 NeuronCores.

Strategy
--------
The KAN grid is uniform (knots -2.2:0.4:2.2) and x lies in [0,1), so every
B-spline basis value B_j(x) is an exact linear combination of 6 "truncated
power" features of x:  [1, x, x^2, x^3, relu(x-0.2)^3, relu(x-0.6)^3].
Folding that j-recombination into the (constant) weights turns

    out = silu(x) @ Wb.T + B(x).reshape @ (Ws*s).reshape.T      (K = 1024+8192)

into

    out = sum_f feat_f(x) @ Vf + bias                           (K = 6*1024)

with feat = [silu(x), x, x^2, x^3, r1^3, r2^3].  The Vf / bias recombination
is an exact (f64) reparameterization of the weights, done once on the host.

Device kernel (per core, data-parallel over batch: 1024 rows/core):
  - DMA x in natural layout (contiguous 4KB/partition), transpose 128x128
    tiles on the PE (feature dim -> partitions),
  - compute the 6 features elementwise on ACT/DVE into fp16 SBUF tiles,
  - K=6144 fp16 matmul with f32 PSUM accumulation, psum = (batch, out):
    lhsT = feature slices, rhs = weight tiles (both DMA-natural),
  - add bias on psum eviction (DVE), natural-layout output store.
"""

import numpy as np
from contextlib import ExitStack

import concourse.bass as bass
import concourse.mybir as mybir
import concourse.tile as tile
from concourse import bacc
from concourse.bass_utils import run_bass_kernel_spmd
from concourse.masks import make_identity

P = 128
N_CORES = 8
N_FULL = 8192
D_IN = 1024
D_OUT = 1024
NB = N_FULL // N_CORES          # 1024 batch rows per core
NF = 6                          # feature count
IB = D_IN // P                  # 8 i-blocks
BB = NB // P                    # 8 batch blocks
NK = IB * NF                    # 48 accumulation steps

F32 = mybir.dt.float32
F16 = mybir.dt.float16
AF = mybir.ActivationFunctionType

# exact B-spline -> truncated-power coefficients (rows: 1, x, x^2, x^3,
# relu(x-.2)^3, relu(x-.6)^3; cols: j=0..7), all exact multiples of 1/48
_C48 = np.array([
    [0, 0,    1,   23,   23,    1,    0,   0],
    [0, 0,  -15,  -75,   75,   15,    0,   0],
    [0, 0,   75,  -75,  -75,   75,    0,   0],
    [0, 0, -125,  375, -375,  125,    0,   0],
    [0, 0,  125, -500,  750, -500,  125,   0],
    [0, 0,    0,  125, -500,  750, -500, 125],
], dtype=np.float64) / 48.0


def _build_bass():
    nc = bacc.Bacc(None, target_bir_lowering=False, debug=False)
    xs = nc.declare_dram_parameter("xs", [NB, D_IN], F32, isOutput=False)
    wf = nc.declare_dram_parameter("wf", [NF, D_IN, D_OUT], F16, isOutput=False)
    biasr = nc.declare_dram_parameter("biasr", [P, D_OUT], F32, isOutput=False)
    out = nc.declare_dram_parameter("out", [NB, D_OUT], F32, isOutput=True)

    with tile.TileContext(nc) as tc, ExitStack() as ctx:
        xpool = ctx.enter_context(tc.tile_pool(name="xp", bufs=1))
        xtp = ctx.enter_context(tc.tile_pool(name="xtp", bufs=2))
        fpool = ctx.enter_context(tc.tile_pool(name="fp", bufs=1))
        tpool = ctx.enter_context(tc.tile_pool(name="tp", bufs=1))
        wpool = ctx.enter_context(tc.tile_pool(name="wp", bufs=1))
        pspool = ctx.enter_context(tc.tile_pool(name="ps", bufs=1, space="PSUM"))
        opool = ctx.enter_context(tc.tile_pool(name="op", bufs=1))
        bpool = ctx.enter_context(tc.tile_pool(name="bp", bufs=1))

        bias_sb = bpool.tile([P, D_OUT], F32, tag="bias", name="bias_sb")
        nc.sync.dma_start(out=bias_sb[:], in_=biasr[:])
        ident = bpool.tile([P, P], F32, tag="ident", name="ident")
        make_identity(nc, ident[:])
        shift_ap = {}
        for sh in (-0.2, -0.6):
            shtile = bpool.tile([P, 1], F32, tag=f"sh{sh}", name=f"sh{sh}")
            nc.vector.memset(shtile[:], sh)
            shift_ap[sh] = shtile

        # ---- load x natural-layout, transpose on PE to (i, b) tiles ----
        xT = {}
        for ib in range(IB):
            xT[ib] = xtp.tile([P, NB], F32, tag=f"xT{ib}", name=f"xT{ib}")
        # stream batch-blocks; transpose each one's 8 column blocks
        for bb in range(BB):
            xb = xpool.tile([P, D_IN], F32, tag=f"xb{bb % 2}", name=f"xb{bb}")
            nc.sync.dma_start(out=xb[:], in_=xs[bb * P:(bb + 1) * P, :])
            for ib in range(IB):
                pt = pspool.tile([P, P], F32, tag=f"ps{(bb * IB + ib) % 8}",
                                 name=f"pst{bb}_{ib}")
                nc.tensor.transpose(pt[:], xb[:, ib * P:(ib + 1) * P],
                                    ident[:])
                nc.scalar.activation(xT[ib][:, bb * P:(bb + 1) * P], pt[:],
                                     AF.Copy)

        feat = {}
        for ib in range(IB):
            xt = xT[ib]
            fs = [fpool.tile([P, NB], F16, tag=f"f{ib}_{f}", name=f"f{ib}_{f}")
                  for f in range(NF)]
            # f0 = silu(x) = x * sigmoid(x), f1 = x (fp16 cast)
            sig = tpool.tile([P, NB], F32, tag="sig", name=f"sig{ib}")
            nc.scalar.activation(sig[:], xt[:], AF.Sigmoid)
            nc.vector.tensor_mul(fs[0][:], sig[:], xt[:])
            nc.scalar.activation(fs[1][:], xt[:], AF.Copy)
            # f2 = x^2, f3 = x^3  (x2 written+read by DVE only)
            x2 = tpool.tile([P, NB], F32, tag="x2", name=f"x2_{ib}")
            nc.vector.tensor_mul(x2[:], xt[:], xt[:])
            nc.vector.tensor_copy(fs[2][:], x2[:])
            nc.vector.tensor_mul(fs[3][:], x2[:], xt[:])
            # f4 = relu(x-0.2)^3, f5 = relu(x-0.6)^3
            for f, sh in ((4, -0.2), (5, -0.6)):
                r = tpool.tile([P, NB], F32, tag=f"r{f}", name=f"r{f}_{ib}")
                nc.scalar.activation(r[:], xt[:], AF.Relu, bias=shift_ap[sh][:])
                rsq = tpool.tile([P, NB], F32, tag=f"rsq{f}", name=f"rsq{f}_{ib}")
                nc.vector.tensor_mul(rsq[:], r[:], r[:])
                nc.vector.tensor_mul(fs[f][:], rsq[:], r[:])
            feat[ib] = fs

        # ---- main matmul: 2 passes over out-halves, psum = (batch, out) ----
        for oh in range(2):
            osl = slice(oh * 512, (oh + 1) * 512)
            ps = [pspool.tile([P, 512], F32, tag=f"ps{bt}",
                              name=f"ps{oh}_{bt}") for bt in range(BB)]

            for ib in range(IB):
                for f in range(NF):
                    k = ib * NF + f
                    w = wpool.tile([P, 512], F16, tag=f"w{k % 8}",
                                   name=f"w{oh}_{ib}_{f}")
                    nc.sync.dma_start(
                        out=w[:], in_=wf[f, ib * P:(ib + 1) * P, osl])
                    for bt in range(BB):
                        nc.tensor.matmul(
                            ps[bt][:],
                            lhsT=feat[ib][f][:, bt * P:(bt + 1) * P],
                            rhs=w[:],
                            start=(k == 0), stop=(k == NK - 1))

            for bt in range(BB):
                osb = opool.tile([P, 512], F32, tag="osb",
                                 name=f"o{oh}_{bt}")
                nc.vector.tensor_add(osb[:], ps[bt][:], bias_sb[:, osl])
                nc.sync.dma_start(out=out[bt * P:(bt + 1) * P, osl],
                                  in_=osb[:])
    nc.compile()
    return nc


def _host_prep(base_weight, spline_weight, spline_scaler):
    S = spline_weight.astype(np.float64) * spline_scaler.astype(np.float64)[..., None]
    bias = np.einsum('oij,j->o', S, _C48[0])
    V = np.einsum('oij,fj->fio', S, _C48[1:], optimize=True)        # (5,i,o)
    wf = np.concatenate([base_weight.astype(np.float64).T[None], V], axis=0)
    wf = np.ascontiguousarray(wf).astype(np.float16)                # (6,i,o)
    biasr = np.ascontiguousarray(
        np.broadcast_to(bias.astype(np.float32)[None, :], (P, D_OUT)))
    return wf, biasr


_RUN_KWARGS = {}   # test-only hook (e.g. trace=True); harness leaves it empty
_LAST = [None]


def kernel(x, grid, base_weight, spline_weight, spline_scaler):
    x = np.ascontiguousarray(np.asarray(x, dtype=np.float32))
    wf, biasr = _host_prep(np.asarray(base_weight), np.asarray(spline_weight),
                           np.asarray(spline_scaler))
    nc = _build_bass()
    in_maps = [{"xs": np.ascontiguousarray(x[c * NB:(c + 1) * NB]),
                "wf": wf, "biasr": biasr} for c in range(N_CORES)]
    res = run_bass_kernel_spmd(nc, in_maps, list(range(N_CORES)), **_RUN_KWARGS)
    _LAST[0] = res
    return np.concatenate([res.results[c]["out"] for c in range(N_CORES)], axis=0)



# revision 5
# speedup vs baseline: 1.3415x; 1.3415x over previous
"""KANLinear forward on 8 Trainium2 NeuronCores.

Strategy
--------
The KAN grid is uniform (knots -2.2:0.4:2.2) and x lies in [0,1), so every
B-spline basis value B_j(x) is an exact linear combination of 6 "truncated
power" features of x:  [1, x, x^2, x^3, relu(x-0.2)^3, relu(x-0.6)^3].
Folding that j-recombination into the (constant) weights turns

    out = silu(x) @ Wb.T + B(x).reshape @ (Ws*s).reshape.T      (K = 1024+8192)

into

    out = sum_f feat_f(x) @ Vf + bias                           (K = 6*1024)

with feat = [silu(x), x, x^2, x^3, r1^3, r2^3].  The Vf / bias recombination
is an exact (f64) reparameterization of the weights, done once on the host.

Device kernel (per core, data-parallel over batch: 1024 rows/core):
  - DMA x natural layout, transpose 128x128 tiles on the PE, evict psum
    to fp16 (feature dim -> partitions).  The evicted xT tile IS feature
    f1; f2 = Square(xT) on ACT; f0 = Silu(xT) on ACT; f3..f5 via ACT
    Relu + DVE multiplies.  All features fp16.
  - K=6144 fp16 matmul, f32 PSUM, psum = (batch, out).  Two half-batch
    passes of 4 batch tiles x 2 out-halves = 8 PSUM banks; each
    stationary (feature) tile feeds the 2 out-half matmuls back-to-back
    to halve LDWEIGHTS traffic.  Weights stream as [128,1024] fp16 tiles
    (re-fetched per half-pass; DMA fully hidden under PE).
  - bias added on psum eviction (DVE), natural-layout output store.
"""

import numpy as np
from contextlib import ExitStack

import concourse.bass as bass
import concourse.mybir as mybir
import concourse.tile as tile
from concourse import bacc
from concourse.bass_utils import run_bass_kernel_spmd
from concourse.masks import make_identity

P = 128
N_CORES = 8
N_FULL = 8192
D_IN = 1024
D_OUT = 1024
NB = N_FULL // N_CORES          # 1024 batch rows per core
NF = 6                          # feature count
IB = D_IN // P                  # 8 i-blocks
BB = NB // P                    # 8 batch blocks
NK = IB * NF                    # 48 accumulation steps

F32 = mybir.dt.float32
F16 = mybir.dt.float16
AF = mybir.ActivationFunctionType

# exact B-spline -> truncated-power coefficients (rows: 1, x, x^2, x^3,
# relu(x-.2)^3, relu(x-.6)^3; cols: j=0..7), all exact multiples of 1/48
_C48 = np.array([
    [0, 0,    1,   23,   23,    1,    0,   0],
    [0, 0,  -15,  -75,   75,   15,    0,   0],
    [0, 0,   75,  -75,  -75,   75,    0,   0],
    [0, 0, -125,  375, -375,  125,    0,   0],
    [0, 0,  125, -500,  750, -500,  125,   0],
    [0, 0,    0,  125, -500,  750, -500, 125],
], dtype=np.float64) / 48.0


def _build_bass():
    nc = bacc.Bacc(None, target_bir_lowering=False, debug=False)
    xs = nc.declare_dram_parameter("xs", [NB, D_IN], F32, isOutput=False)
    wf = nc.declare_dram_parameter("wf", [NF, D_IN, D_OUT], F16, isOutput=False)
    biasr = nc.declare_dram_parameter("biasr", [P, D_OUT], F32, isOutput=False)
    out = nc.declare_dram_parameter("out", [NB, D_OUT], F32, isOutput=True)

    with tile.TileContext(nc) as tc, ExitStack() as ctx:
        xpool = ctx.enter_context(tc.tile_pool(name="xp", bufs=1))
        xtp = ctx.enter_context(tc.tile_pool(name="xtp", bufs=1))
        fpool = ctx.enter_context(tc.tile_pool(name="fp", bufs=1))
        tpool = ctx.enter_context(tc.tile_pool(name="tp", bufs=2))
        wpool = ctx.enter_context(tc.tile_pool(name="wp", bufs=1))
        pspool = ctx.enter_context(tc.tile_pool(name="ps", bufs=1, space="PSUM"))
        opool = ctx.enter_context(tc.tile_pool(name="op", bufs=1))
        bpool = ctx.enter_context(tc.tile_pool(name="bp", bufs=1))

        bias_sb = bpool.tile([P, D_OUT], F32, tag="bias", name="bias_sb")
        nc.sync.dma_start(out=bias_sb[:], in_=biasr[:])
        ident = bpool.tile([P, P], F32, tag="ident", name="ident")
        make_identity(nc, ident[:])
        shift_ap = {}
        for sh in (-0.2, -0.6):
            shtile = bpool.tile([P, 1], F32, tag=f"sh{sh}", name=f"sh{sh}")
            nc.vector.memset(shtile[:], sh)
            shift_ap[sh] = shtile

        # ---- load x natural-layout, transpose on PE to (i, b) fp16 ----
        # xT[ib] doubles as feature f1 (= x).
        xT = {}
        for ib in range(IB):
            xT[ib] = xtp.tile([P, NB], F16, tag=f"xT{ib}", name=f"xT{ib}")
        for bb in range(BB):
            xb = xpool.tile([P, D_IN], F32, tag=f"xb{bb % 2}", name=f"xb{bb}")
            nc.sync.dma_start(out=xb[:], in_=xs[bb * P:(bb + 1) * P, :])
            for ib in range(IB):
                pt = pspool.tile([P, P], F32, tag=f"ps{(bb * IB + ib) % 8}",
                                 name=f"pst{bb}_{ib}")
                nc.tensor.transpose(pt[:], xb[:, ib * P:(ib + 1) * P],
                                    ident[:])
                nc.scalar.activation(xT[ib][:, bb * P:(bb + 1) * P], pt[:],
                                     AF.Copy)

        # ---- features, fp16: feat[ib] = [f0, f1(=xT), f2, f3, f4, f5] ----
        feat = {}
        for ib in range(IB):
            xt = xT[ib]
            f0 = fpool.tile([P, NB], F16, tag=f"f{ib}_0", name=f"f{ib}_0")
            f2 = fpool.tile([P, NB], F16, tag=f"f{ib}_2", name=f"f{ib}_2")
            f3 = fpool.tile([P, NB], F16, tag=f"f{ib}_3", name=f"f{ib}_3")
            f4 = fpool.tile([P, NB], F16, tag=f"f{ib}_4", name=f"f{ib}_4")
            f5 = fpool.tile([P, NB], F16, tag=f"f{ib}_5", name=f"f{ib}_5")
            nc.scalar.activation(f0[:], xt[:], AF.Silu)
            nc.scalar.activation(f2[:], xt[:], AF.Square)
            nc.vector.tensor_mul(f3[:], f2[:], xt[:])
            for fdst, sh, rt in ((f4, -0.2, "r1"), (f5, -0.6, "r2")):
                r = tpool.tile([P, NB], F16, tag=rt, name=f"{rt}_{ib}")
                nc.scalar.activation(r[:], xt[:], AF.Relu, bias=shift_ap[sh][:])
                rs = tpool.tile([P, NB], F16, tag=rt + "s", name=f"{rt}s_{ib}")
                nc.vector.tensor_mul(rs[:], r[:], r[:])
                nc.vector.tensor_mul(fdst[:], rs[:], r[:])
            feat[ib] = [f0, xt, f2, f3, f4, f5]

        # ---- matmul: 2 half-batch passes; psum = 4 bt x 2 oh banks ----
        # stationary (feature) slice shared by the 2 out-half matmuls.
        for hp in range(2):
            ps = {}
            for bt in range(4):
                for oh in range(2):
                    ps[bt, oh] = pspool.tile(
                        [P, 512], F32, tag=f"ps{bt * 2 + oh}",
                        name=f"ps{hp}_{bt}_{oh}")
            for k in range(NK):
                ib, f = divmod(k, NF)
                w = wpool.tile([P, D_OUT], F16, tag=f"w{k % 6}",
                               name=f"w{hp}_{k}")
                nc.sync.dma_start(out=w[:], in_=wf[f, ib * P:(ib + 1) * P, :])
                for bt in range(4):
                    col = (hp * 4 + bt) * P
                    lhsT = feat[ib][f][:, col:col + P]
                    for oh in range(2):
                        nc.tensor.matmul(
                            ps[bt, oh][:], lhsT=lhsT,
                            rhs=w[:, oh * 512:(oh + 1) * 512],
                            start=(k == 0), stop=(k == NK - 1))
            for bt in range(4):
                for oh in range(2):
                    osl = slice(oh * 512, (oh + 1) * 512)
                    osb = opool.tile([P, 512], F32, tag=f"o{bt * 2 + oh}",
                                     name=f"o{hp}_{bt}_{oh}")
                    nc.vector.tensor_add(osb[:], ps[bt, oh][:],
                                         bias_sb[:, osl])
                    row = (hp * 4 + bt) * P
                    nc.sync.dma_start(out=out[row:row + P, osl], in_=osb[:])
    nc.compile()
    return nc


def _host_prep(base_weight, spline_weight, spline_scaler):
    S = spline_weight.astype(np.float64) * spline_scaler.astype(np.float64)[..., None]
    bias = np.einsum('oij,j->o', S, _C48[0])
    V = np.einsum('oij,fj->fio', S, _C48[1:], optimize=True)        # (5,i,o)
    wf = np.concatenate([base_weight.astype(np.float64).T[None], V], axis=0)
    wf = np.ascontiguousarray(wf).astype(np.float16)                # (6,i,o)
    biasr = np.ascontiguousarray(
        np.broadcast_to(bias.astype(np.float32)[None, :], (P, D_OUT)))
    return wf, biasr


_RUN_KWARGS = {}   # test-only hook (e.g. trace=True); harness leaves it empty
_LAST = [None]


def kernel(x, grid, base_weight, spline_weight, spline_scaler):
    x = np.ascontiguousarray(np.asarray(x, dtype=np.float32))
    wf, biasr = _host_prep(np.asarray(base_weight), np.asarray(spline_weight),
                           np.asarray(spline_scaler))
    nc = _build_bass()
    in_maps = [{"xs": np.ascontiguousarray(x[c * NB:(c + 1) * NB]),
                "wf": wf, "biasr": biasr} for c in range(N_CORES)]
    res = run_bass_kernel_spmd(nc, in_maps, list(range(N_CORES)), **_RUN_KWARGS)
    _LAST[0] = res
    return np.concatenate([res.results[c]["out"] for c in range(N_CORES)], axis=0)


# revision 6
# speedup vs baseline: 1.4085x; 1.0500x over previous
"""KANLinear forward on 8 Trainium2 NeuronCores.

Strategy
--------
The KAN grid is uniform (knots -2.2:0.4:2.2) and x lies in [0,1), so every
B-spline basis value B_j(x) is an exact linear combination of 6 "truncated
power" features of x:  [1, x, x^2, x^3, relu(x-0.2)^3, relu(x-0.6)^3].
Folding that j-recombination into the (constant) weights turns

    out = silu(x) @ Wb.T + B(x).reshape @ (Ws*s).reshape.T      (K = 1024+8192)

into

    out = sum_f feat_f(x) @ Vf + bias                           (K = 6*1024)

with feat = [silu(x), x, x^2, x^3, r1^3, r2^3].  The Vf / bias recombination
is an exact (f64) reparameterization of the weights, done once on the host.

Device kernel (per core, data-parallel over batch: 1024 rows/core):
  - x is pre-cast to fp16 on the host; the DMA XBAR transpose engine
    lands x^T (feature dim -> partitions) directly in SBUF: no PE
    transposes, no PSUM staging, no eviction copies.
  - The x^T tile IS feature f1; f0 = Silu(x^T) and f2 = Square(x^T) on
    ACT; f3..f5 via ACT Relu + DVE multiplies.  All features fp16.
  - K=6144 fp16 matmul with f32 PSUM accumulation, psum = (batch, out):
    2 passes over out-halves, 8 batch-tile PSUM banks each.
  - bias added on psum eviction (DVE), natural-layout output store.
"""

import numpy as np
from contextlib import ExitStack

import concourse.bass as bass
import concourse.mybir as mybir
import concourse.tile as tile
from concourse import bacc
from concourse.bass_utils import run_bass_kernel_spmd

P = 128
N_CORES = 8
N_FULL = 8192
D_IN = 1024
D_OUT = 1024
NB = N_FULL // N_CORES          # 1024 batch rows per core
NF = 6                          # feature count
IB = D_IN // P                  # 8 i-blocks
BB = NB // P                    # 8 batch blocks
NK = IB * NF                    # 48 accumulation steps

F32 = mybir.dt.float32
F16 = mybir.dt.float16
AF = mybir.ActivationFunctionType

# exact B-spline -> truncated-power coefficients (rows: 1, x, x^2, x^3,
# relu(x-.2)^3, relu(x-.6)^3; cols: j=0..7), all exact multiples of 1/48
_C48 = np.array([
    [0, 0,    1,   23,   23,    1,    0,   0],
    [0, 0,  -15,  -75,   75,   15,    0,   0],
    [0, 0,   75,  -75,  -75,   75,    0,   0],
    [0, 0, -125,  375, -375,  125,    0,   0],
    [0, 0,  125, -500,  750, -500,  125,   0],
    [0, 0,    0,  125, -500,  750, -500, 125],
], dtype=np.float64) / 48.0


def _build_bass():
    nc = bacc.Bacc(None, target_bir_lowering=False, debug=False)
    xs16 = nc.declare_dram_parameter("xs16", [NB, D_IN], F16, isOutput=False)
    wf = nc.declare_dram_parameter("wf", [NF, D_IN, D_OUT], F16, isOutput=False)
    biasr = nc.declare_dram_parameter("biasr", [P, D_OUT], F32, isOutput=False)
    out = nc.declare_dram_parameter("out", [NB, D_OUT], F32, isOutput=True)

    with tile.TileContext(nc) as tc, ExitStack() as ctx:
        xtp = ctx.enter_context(tc.tile_pool(name="xtp", bufs=1))
        fpool = ctx.enter_context(tc.tile_pool(name="fp", bufs=1))
        tpool = ctx.enter_context(tc.tile_pool(name="tp", bufs=2))
        wpool = ctx.enter_context(tc.tile_pool(name="wp", bufs=1))
        pspool = ctx.enter_context(tc.tile_pool(name="ps", bufs=1, space="PSUM"))
        opool = ctx.enter_context(tc.tile_pool(name="op", bufs=1))
        bpool = ctx.enter_context(tc.tile_pool(name="bp", bufs=1))

        bias_sb = bpool.tile([P, D_OUT], F32, tag="bias", name="bias_sb")
        nc.sync.dma_start(out=bias_sb[:], in_=biasr[:])
        shift_ap = {}
        for sh in (-0.2, -0.6):
            shtile = bpool.tile([P, 1], F32, tag=f"sh{sh}", name=f"sh{sh}")
            nc.vector.memset(shtile[:], sh)
            shift_ap[sh] = shtile

        # ---- x^T straight from HBM via the DMA XBAR transpose ----
        xT = {}
        for ib in range(IB):
            xT[ib] = xtp.tile([P, NB], F16, tag=f"xT{ib}", name=f"xT{ib}")
            nc.sync.dma_start(out=xT[ib][:], in_=xs16[:, ib * P:(ib + 1) * P],
                              transpose=True)

        # ---- features, fp16: feat[ib] = [f0, f1(=xT), f2, f3, f4, f5] ----
        feat = {}
        for ib in range(IB):
            xt = xT[ib]
            f0 = fpool.tile([P, NB], F16, tag=f"f{ib}_0", name=f"f{ib}_0")
            f2 = fpool.tile([P, NB], F16, tag=f"f{ib}_2", name=f"f{ib}_2")
            f3 = fpool.tile([P, NB], F16, tag=f"f{ib}_3", name=f"f{ib}_3")
            f4 = fpool.tile([P, NB], F16, tag=f"f{ib}_4", name=f"f{ib}_4")
            f5 = fpool.tile([P, NB], F16, tag=f"f{ib}_5", name=f"f{ib}_5")
            nc.scalar.activation(f0[:], xt[:], AF.Silu)
            nc.scalar.activation(f2[:], xt[:], AF.Square)
            nc.vector.tensor_mul(f3[:], f2[:], xt[:])
            for fdst, sh, rt in ((f4, -0.2, "r1"), (f5, -0.6, "r2")):
                r = tpool.tile([P, NB], F16, tag=rt, name=f"{rt}_{ib}")
                nc.scalar.activation(r[:], xt[:], AF.Relu, bias=shift_ap[sh][:])
                rs = tpool.tile([P, NB], F16, tag=rt + "s", name=f"{rt}s_{ib}")
                nc.vector.tensor_mul(rs[:], r[:], r[:])
                nc.vector.tensor_mul(fdst[:], rs[:], r[:])
            feat[ib] = [f0, xt, f2, f3, f4, f5]

        # ---- main matmul: 2 passes over out-halves, psum = (batch, out) ----
        for oh in range(2):
            osl = slice(oh * 512, (oh + 1) * 512)
            ps = [pspool.tile([P, 512], F32, tag=f"ps{bt}",
                              name=f"ps{oh}_{bt}") for bt in range(BB)]
            for k in range(NK):
                ib, f = divmod(k, NF)
                w = wpool.tile([P, 512], F16, tag=f"w{k % 8}",
                               name=f"w{oh}_{k}")
                nc.sync.dma_start(out=w[:], in_=wf[f, ib * P:(ib + 1) * P, osl])
                for bt in range(BB):
                    nc.tensor.matmul(
                        ps[bt][:],
                        lhsT=feat[ib][f][:, bt * P:(bt + 1) * P],
                        rhs=w[:],
                        start=(k == 0), stop=(k == NK - 1))
            for bt in range(BB):
                osb = opool.tile([P, 512], F32, tag=f"o{bt}",
                                 name=f"o{oh}_{bt}")
                nc.vector.tensor_add(osb[:], ps[bt][:], bias_sb[:, osl])
                nc.sync.dma_start(out=out[bt * P:(bt + 1) * P, osl],
                                  in_=osb[:])
    nc.compile()
    return nc


def _host_prep(base_weight, spline_weight, spline_scaler):
    S = spline_weight.astype(np.float64) * spline_scaler.astype(np.float64)[..., None]
    bias = np.einsum('oij,j->o', S, _C48[0])
    V = np.einsum('oij,fj->fio', S, _C48[1:], optimize=True)        # (5,i,o)
    wf = np.concatenate([base_weight.astype(np.float64).T[None], V], axis=0)
    wf = np.ascontiguousarray(wf).astype(np.float16)                # (6,i,o)
    biasr = np.ascontiguousarray(
        np.broadcast_to(bias.astype(np.float32)[None, :], (P, D_OUT)))
    return wf, biasr


_RUN_KWARGS = {}   # test-only hook (e.g. trace=True); harness leaves it empty
_LAST = [None]


def kernel(x, grid, base_weight, spline_weight, spline_scaler):
    x16 = np.ascontiguousarray(np.asarray(x).astype(np.float16))
    wf, biasr = _host_prep(np.asarray(base_weight), np.asarray(spline_weight),
                           np.asarray(spline_scaler))
    nc = _build_bass()
    in_maps = [{"xs16": np.ascontiguousarray(x16[c * NB:(c + 1) * NB]),
                "wf": wf, "biasr": biasr} for c in range(N_CORES)]
    res = run_bass_kernel_spmd(nc, in_maps, list(range(N_CORES)), **_RUN_KWARGS)
    _LAST[0] = res
    return np.concatenate([res.results[c]["out"] for c in range(N_CORES)], axis=0)


# revision 8
# speedup vs baseline: 1.4132x; 1.0033x over previous
"""KANLinear forward on 8 Trainium2 NeuronCores.

Strategy
--------
The KAN grid is uniform (knots -2.2:0.4:2.2) and x lies in [0,1), so every
B-spline basis value B_j(x) is an exact linear combination of 6 "truncated
power" features of x:  [1, x, x^2, x^3, relu(x-0.2)^3, relu(x-0.6)^3].
Folding that j-recombination into the (constant) weights turns

    out = silu(x) @ Wb.T + B(x).reshape @ (Ws*s).reshape.T      (K = 1024+8192)

into

    out = sum_f feat_f(x) @ Vf + bias                           (K = 6*1024)

with feat = [silu(x), x, x^2, x^3, r1^3, r2^3].  The Vf / bias recombination
is an exact (f64) reparameterization of the weights, done once on the host.

Device kernel (per core, data-parallel over batch: 1024 rows/core):
  - x is pre-cast to fp16 on the host; the DMA XBAR transpose engine
    lands x^T (feature dim -> partitions) directly in SBUF: no PE
    transposes, no PSUM staging, no eviction copies.  First x^T tile is
    issued on the SP queue ahead of the weights; the other 7 issue from
    the ACT queue interleaved with feature ops so the SP queue can start
    streaming weights immediately.
  - The x^T tile IS feature f1; f0 = Silu(x^T), f2 = Square(x^T) on ACT;
    r1/r2 = fused (x-a).max(0) on DVE; f3..f5 DVE multiplies. All fp16.
  - K=6144 fp16 matmul with f32 PSUM accumulation, psum = (batch, out):
    2 passes over out-halves, 8 batch-tile PSUM banks each.  Weights are
    host-packed into 1 MB groups of 8 K-steps so one DMA issue covers 8
    matmul steps (12 issues total).
  - bias added on psum eviction (DVE + GPSIMD), output DMAs issue from
    the GPSIMD queue to keep the tail off the busy SP queue.
"""

import numpy as np
from contextlib import ExitStack

import concourse.bass as bass
import concourse.mybir as mybir
import concourse.tile as tile
from concourse import bacc
from concourse.bass_utils import run_bass_kernel_spmd

P = 128
N_CORES = 8
N_FULL = 8192
D_IN = 1024
D_OUT = 1024
NB = N_FULL // N_CORES          # 1024 batch rows per core
NF = 6                          # feature count
IB = D_IN // P                  # 8 i-blocks
BB = NB // P                    # 8 batch blocks
NK = IB * NF                    # 48 accumulation steps
KG = 8                          # K-steps per weight DMA group
NG = NK // KG                   # 6 groups per out-half

F32 = mybir.dt.float32
F16 = mybir.dt.float16
AF = mybir.ActivationFunctionType
ALU = mybir.AluOpType

# exact B-spline -> truncated-power coefficients (rows: 1, x, x^2, x^3,
# relu(x-.2)^3, relu(x-.6)^3; cols: j=0..7), all exact multiples of 1/48
_C48 = np.array([
    [0, 0,    1,   23,   23,    1,    0,   0],
    [0, 0,  -15,  -75,   75,   15,    0,   0],
    [0, 0,   75,  -75,  -75,   75,    0,   0],
    [0, 0, -125,  375, -375,  125,    0,   0],
    [0, 0,  125, -500,  750, -500,  125,   0],
    [0, 0,    0,  125, -500,  750, -500, 125],
], dtype=np.float64) / 48.0


def _build_bass():
    nc = bacc.Bacc(None, target_bir_lowering=False, debug=False)
    xs16 = nc.declare_dram_parameter("xs16", [NB, D_IN], F16, isOutput=False)
    # weights packed as [oh][group][partition][KG*512] (see _host_prep)
    wg = nc.declare_dram_parameter("wg", [2, NG, P, KG * 512], F16,
                                   isOutput=False)
    biasr = nc.declare_dram_parameter("biasr", [P, D_OUT], F32, isOutput=False)
    out = nc.declare_dram_parameter("out", [NB, D_OUT], F32, isOutput=True)

    with tile.TileContext(nc) as tc, ExitStack() as ctx:
        xtp = ctx.enter_context(tc.tile_pool(name="xtp", bufs=1))
        fpool = ctx.enter_context(tc.tile_pool(name="fp", bufs=1))
        tpool = ctx.enter_context(tc.tile_pool(name="tp", bufs=2))
        wpool = ctx.enter_context(tc.tile_pool(name="wp", bufs=1))
        pspool = ctx.enter_context(tc.tile_pool(name="ps", bufs=1, space="PSUM"))
        opool = ctx.enter_context(tc.tile_pool(name="op", bufs=1))
        bpool = ctx.enter_context(tc.tile_pool(name="bp", bufs=1))

        bias_sb = bpool.tile([P, D_OUT], F32, tag="bias", name="bias_sb")
        nc.gpsimd.dma_start(out=bias_sb[:], in_=biasr[:])

        xT = {}
        for ib in range(IB):
            xT[ib] = xtp.tile([P, NB], F16, tag=f"xT{ib}", name=f"xT{ib}")
        # first x^T tile on the SP queue, ahead of the weight streams
        nc.sync.dma_start(out=xT[0][:], in_=xs16[:, 0:P], transpose=True)

        # ---- features, fp16: feat[ib] = [f0, f1(=xT), f2, f3, f4, f5] ----
        feat = {}
        for ib in range(IB):
            xt = xT[ib]
            if ib + 1 < IB:   # prefetch next x^T from the ACT queue
                nc.scalar.dma_start(out=xT[ib + 1][:],
                                    in_=xs16[:, (ib + 1) * P:(ib + 2) * P],
                                    transpose=True)
            f0 = fpool.tile([P, NB], F16, tag=f"f{ib}_0", name=f"f{ib}_0")
            f2 = fpool.tile([P, NB], F16, tag=f"f{ib}_2", name=f"f{ib}_2")
            f3 = fpool.tile([P, NB], F16, tag=f"f{ib}_3", name=f"f{ib}_3")
            f4 = fpool.tile([P, NB], F16, tag=f"f{ib}_4", name=f"f{ib}_4")
            f5 = fpool.tile([P, NB], F16, tag=f"f{ib}_5", name=f"f{ib}_5")
            nc.scalar.activation(f0[:], xt[:], AF.Silu)
            nc.scalar.activation(f2[:], xt[:], AF.Square)
            nc.vector.tensor_mul(f3[:], f2[:], xt[:])
            for fdst, sh, rt in ((f4, -0.2, "r1"), (f5, -0.6, "r2")):
                r = tpool.tile([P, NB], F16, tag=rt, name=f"{rt}_{ib}")
                nc.vector.tensor_scalar(r[:], xt[:], sh, 0.0,
                                        ALU.add, ALU.max)
                rs = tpool.tile([P, NB], F16, tag=rt + "s", name=f"{rt}s_{ib}")
                nc.vector.tensor_mul(rs[:], r[:], r[:])
                nc.vector.tensor_mul(fdst[:], rs[:], r[:])
            feat[ib] = [f0, xt, f2, f3, f4, f5]

        # ---- main matmul: 2 passes over out-halves, psum = (batch, out) ----
        for oh in range(2):
            osl = slice(oh * 512, (oh + 1) * 512)
            ps = [pspool.tile([P, 512], F32, tag=f"ps{bt}",
                              name=f"ps{oh}_{bt}") for bt in range(BB)]
            for g in range(NG):
                w = wpool.tile([P, KG * 512], F16, tag=f"w{g % 3}",
                               name=f"w{oh}_{g}")
                nc.sync.dma_start(out=w[:], in_=wg[oh, g])
                for j in range(KG):
                    k = g * KG + j
                    ib, f = divmod(k, NF)
                    rhs = w[:, j * 512:(j + 1) * 512]
                    for bt in range(BB):
                        nc.tensor.matmul(
                            ps[bt][:],
                            lhsT=feat[ib][f][:, bt * P:(bt + 1) * P],
                            rhs=rhs,
                            start=(k == 0), stop=(k == NK - 1))
            for bt in range(BB):
                osb = opool.tile([P, 512], F32, tag=f"o{bt}",
                                 name=f"o{oh}_{bt}")
                nc.vector.tensor_add(osb[:], ps[bt][:], bias_sb[:, osl])
                nc.gpsimd.dma_start(out=out[bt * P:(bt + 1) * P, osl],
                                    in_=osb[:])
    nc.compile()
    return nc


def _host_prep(base_weight, spline_weight, spline_scaler):
    S = spline_weight.astype(np.float64) * spline_scaler.astype(np.float64)[..., None]
    bias = np.einsum('oij,j->o', S, _C48[0])
    V = np.einsum('oij,fj->fio', S, _C48[1:], optimize=True)        # (5,i,o)
    wf = np.concatenate([base_weight.astype(np.float64).T[None], V], axis=0)
    wf = np.ascontiguousarray(wf).astype(np.float16)                # (6,i,o)
    # pack weights: wg[oh, g, p, j*512 + c] = wf[f(k), ib(k)*128 + p,
    # oh*512 + c], k = g*KG + j   (one contiguous 1 MB line-group per DMA)
    wk = wf.reshape(NF, IB, P, 2, 512)            # (f, ib, p, oh, c)
    wk = wk.transpose(3, 1, 0, 2, 4)              # (oh, ib, f, p, c)
    wk = wk.reshape(2, NK, P, 512)                # k = ib*NF + f
    wg = np.ascontiguousarray(
        wk.reshape(2, NG, KG, P, 512).transpose(0, 1, 3, 2, 4)
          .reshape(2, NG, P, KG * 512))
    biasr = np.ascontiguousarray(
        np.broadcast_to(bias.astype(np.float32)[None, :], (P, D_OUT)))
    return wg, biasr


_RUN_KWARGS = {}   # test-only hook (e.g. trace=True); harness leaves it empty
_LAST = [None]


def kernel(x, grid, base_weight, spline_weight, spline_scaler):
    x16 = np.ascontiguousarray(np.asarray(x).astype(np.float16))
    wg, biasr = _host_prep(np.asarray(base_weight), np.asarray(spline_weight),
                           np.asarray(spline_scaler))
    nc = _build_bass()
    in_maps = [{"xs16": np.ascontiguousarray(x16[c * NB:(c + 1) * NB]),
                "wg": wg, "biasr": biasr} for c in range(N_CORES)]
    res = run_bass_kernel_spmd(nc, in_maps, list(range(N_CORES)), **_RUN_KWARGS)
    _LAST[0] = res
    return np.concatenate([res.results[c]["out"] for c in range(N_CORES)], axis=0)


# revision 12
# speedup vs baseline: 1.4575x; 1.0314x over previous
"""KANLinear forward on 8 Trainium2 NeuronCores.

Strategy
--------
The KAN grid is uniform (knots -2.2:0.4:2.2) and x lies in [0,1), so every
B-spline basis value B_j(x) is an exact linear combination of 6 "truncated
power" features of x:  [1, x, x^2, x^3, relu(x-0.2)^3, relu(x-0.6)^3].
Folding that j-recombination into the (constant) weights turns

    out = silu(x) @ Wb.T + B(x).reshape @ (Ws*s).reshape.T      (K = 1024+8192)

into

    out = sum_f feat_f(x) @ Vf + bias                           (K = 6*1024)

with feat = [silu(x), x, x^2, x^3, r1^3, r2^3].  The Vf / bias recombination
is an exact (f64) reparameterization of the weights, done once on the host.

Device kernel (per core, data-parallel over batch: 1024 rows/core):
  - x is pre-cast to fp16 on the host; the DMA XBAR transpose engine
    lands x^T (feature dim -> partitions) directly in SBUF: no PE
    transposes, no PSUM staging, no eviction copies.  First x^T tile is
    issued on the SP queue ahead of the weights; the other 7 issue from
    the ACT queue interleaved with feature ops so the SP queue can start
    streaming weights immediately.
  - The x^T tile IS feature f1; f0 = Silu(x^T), f2 = Square(x^T) on ACT;
    r1/r2 = fused (x-a).max(0) on DVE; f3..f5 DVE multiplies. All fp16.
  - K=6144 fp16 matmul with f32 PSUM accumulation, psum = (batch, out):
    2 passes over out-halves, 8 batch-tile PSUM banks each.  Weights are
    host-packed into 1 MB groups of 8 K-steps so one DMA issue covers 8
    matmul steps (12 issues total).
  - bias added on psum eviction (DVE + GPSIMD), output DMAs issue from
    the GPSIMD queue to keep the tail off the busy SP queue.
"""

import numpy as np
from contextlib import ExitStack

import concourse.bass as bass
import concourse.mybir as mybir
import concourse.tile as tile
from concourse import bacc
from concourse.bass_utils import run_bass_kernel_spmd

P = 128
N_CORES = 8
N_FULL = 8192
D_IN = 1024
D_OUT = 1024
NB = N_FULL // N_CORES          # 1024 batch rows per core
NF = 6                          # feature count
IB = D_IN // P                  # 8 i-blocks
BB = NB // P                    # 8 batch blocks
NK = IB * NF                    # 48 accumulation steps
KG = 8                          # K-steps per weight DMA group
NG = NK // KG                   # 6 groups per out-half

F32 = mybir.dt.float32
F16 = mybir.dt.float16
AF = mybir.ActivationFunctionType
ALU = mybir.AluOpType

# exact B-spline -> truncated-power coefficients (rows: 1, x, x^2, x^3,
# relu(x-.2)^3, relu(x-.6)^3; cols: j=0..7), all exact multiples of 1/48
_C48 = np.array([
    [0, 0,    1,   23,   23,    1,    0,   0],
    [0, 0,  -15,  -75,   75,   15,    0,   0],
    [0, 0,   75,  -75,  -75,   75,    0,   0],
    [0, 0, -125,  375, -375,  125,    0,   0],
    [0, 0,  125, -500,  750, -500,  125,   0],
    [0, 0,    0,  125, -500,  750, -500, 125],
], dtype=np.float64) / 48.0


def _build_bass():
    nc = bacc.Bacc(None, target_bir_lowering=False, debug=False)
    xs16 = nc.declare_dram_parameter("xs16", [NB, D_IN], F16, isOutput=False)
    # weights packed as [oh][group][partition][KG*512] (see _host_prep)
    wg = nc.declare_dram_parameter("wg", [2, NG, P, KG * 512], F16,
                                   isOutput=False)
    biasr = nc.declare_dram_parameter("biasr", [1, D_OUT], F16, isOutput=False)
    out = nc.declare_dram_parameter("out", [NB, D_OUT], F32, isOutput=True)

    with tile.TileContext(nc) as tc, ExitStack() as ctx:
        xtp = ctx.enter_context(tc.tile_pool(name="xtp", bufs=1))
        fpool = ctx.enter_context(tc.tile_pool(name="fp", bufs=1))
        tpool = ctx.enter_context(tc.tile_pool(name="tp", bufs=2))
        wpool = ctx.enter_context(tc.tile_pool(name="wp", bufs=1))
        pspool = ctx.enter_context(tc.tile_pool(name="ps", bufs=1, space="PSUM"))
        opool = ctx.enter_context(tc.tile_pool(name="op", bufs=1))
        bpool = ctx.enter_context(tc.tile_pool(name="bp", bufs=1))

        bias_sb = bpool.tile([1, D_OUT], F16, tag="bias", name="bias_sb")
        nc.gpsimd.dma_start(out=bias_sb[:], in_=biasr[:])
        ones16 = bpool.tile([1, P], F16, tag="ones", name="ones16")
        nc.vector.memset(ones16[:], 1.0)

        # x^T tiles via the DMA XBAR transpose, issued on the SP queue
        # interleaved with the weight-group streams (ACT queue stays free
        # for feature ops so the first matmul isn't starved).
        xT = {}
        for ib in range(IB):
            xT[ib] = xtp.tile([P, NB], F16, tag=f"xT{ib}", name=f"xT{ib}")
        nc.sync.dma_start(out=xT[0][:], in_=xs16[:, 0:P], transpose=True)

        wtiles = {}
        for oh, g in ((0, 0), (0, 1)):
            wtiles[oh, g] = wpool.tile([P, KG * 512], F16, tag=f"w{g % 3}",
                                       name=f"w{oh}_{g}")
            nc.sync.dma_start(out=wtiles[oh, g][:], in_=wg[oh, g])
        for ib in range(1, IB):
            nc.sync.dma_start(out=xT[ib][:],
                              in_=xs16[:, ib * P:(ib + 1) * P],
                              transpose=True)

        # ---- features, fp16: feat[ib] = [f0, f1(=xT), f2, f3, f4, f5] ----
        feat = {}
        for ib in range(IB):
            xt = xT[ib]
            f0 = fpool.tile([P, NB], F16, tag=f"f{ib}_0", name=f"f{ib}_0")
            f2 = fpool.tile([P, NB], F16, tag=f"f{ib}_2", name=f"f{ib}_2")
            f3 = fpool.tile([P, NB], F16, tag=f"f{ib}_3", name=f"f{ib}_3")
            f4 = fpool.tile([P, NB], F16, tag=f"f{ib}_4", name=f"f{ib}_4")
            f5 = fpool.tile([P, NB], F16, tag=f"f{ib}_5", name=f"f{ib}_5")
            nc.scalar.activation(f0[:], xt[:], AF.Silu)
            nc.scalar.activation(f2[:], xt[:], AF.Square)
            nc.vector.tensor_mul(f3[:], f2[:], xt[:])
            for fdst, sh, rt in ((f4, -0.2, "r1"), (f5, -0.6, "r2")):
                r = tpool.tile([P, NB], F16, tag=rt, name=f"{rt}_{ib}")
                nc.vector.tensor_scalar(r[:], xt[:], sh, 0.0,
                                        ALU.add, ALU.max)
                rs = tpool.tile([P, NB], F16, tag=rt + "s", name=f"{rt}s_{ib}")
                nc.vector.tensor_mul(rs[:], r[:], r[:])
                nc.vector.tensor_mul(fdst[:], rs[:], r[:])
            feat[ib] = [f0, xt, f2, f3, f4, f5]

        # ---- main matmul: 2 passes over out-halves, psum = (batch, out) ----
        # bias enters the accumulation as a K=1 matmul (ones^T @ bias_row),
        # so psum eviction is a plain copy (split across ACT and DVE).
        for oh in range(2):
            osl = slice(oh * 512, (oh + 1) * 512)
            ps = [pspool.tile([P, 512], F32, tag=f"ps{bt}",
                              name=f"ps{oh}_{bt}") for bt in range(BB)]
            for bt in range(BB):
                nc.tensor.matmul(ps[bt][:], lhsT=ones16[:],
                                 rhs=bias_sb[:, osl],
                                 start=True, stop=False)
            for g in range(NG):
                if (oh, g) in wtiles:
                    w = wtiles[oh, g]
                else:
                    w = wpool.tile([P, KG * 512], F16, tag=f"w{g % 3}",
                                   name=f"w{oh}_{g}")
                    nc.sync.dma_start(out=w[:], in_=wg[oh, g])
                for j in range(KG):
                    k = g * KG + j
                    ib, f = divmod(k, NF)
                    rhs = w[:, j * 512:(j + 1) * 512]
                    for bt in range(BB):
                        nc.tensor.matmul(
                            ps[bt][:],
                            lhsT=feat[ib][f][:, bt * P:(bt + 1) * P],
                            rhs=rhs,
                            start=False, stop=(k == NK - 1))
            for bt in range(BB):
                osb = opool.tile([P, 512], F32, tag=f"o{bt}",
                                 name=f"o{oh}_{bt}")
                if bt % 2 == 0:
                    nc.scalar.activation(osb[:], ps[bt][:], AF.Copy)
                else:
                    nc.vector.tensor_copy(osb[:], ps[bt][:])
                nc.gpsimd.dma_start(out=out[bt * P:(bt + 1) * P, osl],
                                    in_=osb[:])
    nc.compile()
    return nc


def _host_prep(base_weight, spline_weight, spline_scaler):
    S = spline_weight.astype(np.float64) * spline_scaler.astype(np.float64)[..., None]
    bias = np.einsum('oij,j->o', S, _C48[0])
    V = np.einsum('oij,fj->fio', S, _C48[1:], optimize=True)        # (5,i,o)
    wf = np.concatenate([base_weight.astype(np.float64).T[None], V], axis=0)
    wf = np.ascontiguousarray(wf).astype(np.float16)                # (6,i,o)
    # pack weights: wg[oh, g, p, j*512 + c] = wf[f(k), ib(k)*128 + p,
    # oh*512 + c], k = g*KG + j   (one contiguous 1 MB line-group per DMA)
    wk = wf.reshape(NF, IB, P, 2, 512)            # (f, ib, p, oh, c)
    wk = wk.transpose(3, 1, 0, 2, 4)              # (oh, ib, f, p, c)
    wk = wk.reshape(2, NK, P, 512)                # k = ib*NF + f
    wg = np.ascontiguousarray(
        wk.reshape(2, NG, KG, P, 512).transpose(0, 1, 3, 2, 4)
          .reshape(2, NG, P, KG * 512))
    biasr = np.ascontiguousarray(bias.astype(np.float16)[None, :])
    return wg, biasr


_RUN_KWARGS = {}   # test-only hook (e.g. trace=True); harness leaves it empty
_LAST = [None]


def kernel(x, grid, base_weight, spline_weight, spline_scaler):
    x16 = np.ascontiguousarray(np.asarray(x).astype(np.float16))
    wg, biasr = _host_prep(np.asarray(base_weight), np.asarray(spline_weight),
                           np.asarray(spline_scaler))
    nc = _build_bass()
    in_maps = [{"xs16": np.ascontiguousarray(x16[c * NB:(c + 1) * NB]),
                "wg": wg, "biasr": biasr} for c in range(N_CORES)]
    res = run_bass_kernel_spmd(nc, in_maps, list(range(N_CORES)), **_RUN_KWARGS)
    _LAST[0] = res
    return np.concatenate([res.results[c]["out"] for c in range(N_CORES)], axis=0)
